# revision 1
# baseline (speedup 1.0000x reference)
"""Causal single-head attention kernel for TRN2 (one batch element per core).

Computes: out = softmax(causal((X_q Wq + bq)(X_k Wk + bk)^T / sqrt(H))) (X_v Wv + bv)
Shapes per core: Q,K,V [S, E]; Wq/Wk/Wv [E, H]; bq/bk/bv [H]; out [S, H].

Layout strategy (per core):
- Inputs transposed on PE (fp32, exact) so E lands on partitions.
- Projections produce qT/kT/vT [H, S]; fp32r matmuls (1 cyc/row).
- Scores computed transposed: weiT [Sk, Sq-chunk]; exp+scale fused on ACT;
  causal handled by skipping above-diagonal tiles + 0/1 masks on diagonal.
- PV accumulates outT [H, Sq-chunk] in PSUM over Sk tiles; softmax
  denominators via ones-vector matmuls; normalization deferred to the end.
"""

from contextlib import ExitStack

import numpy as np

import concourse.bacc as bacc
import concourse.bass as bass
import concourse.mybir as mybir
import concourse.tile as tile
from concourse.masks import make_identity

F32 = mybir.dt.float32
F32R = mybir.dt.float32r

CH = 512          # Sq chunk width (psum bank)
PT = 128          # partition tile


def build(S=2048, E=1024, H=128, n_cores=8, mm_r=True):
    """Build + compile the Bacc kernel. Returns nc."""
    EC = E // PT              # E chunks (8)
    NCHUNK = S // CH          # Sq chunks (4)
    TPC = CH // PT            # S-tiles per chunk (4)
    scale = float(H) ** -0.5
    MMD = F32R if mm_r else F32   # dtype for matmul operands

    nc = bacc.Bacc("TRN2", target_bir_lowering=False, debug=False,
                   num_devices=n_cores)

    Qd = nc.declare_dram_parameter("Q", [S, E], F32, isOutput=False)
    Kd = nc.declare_dram_parameter("K", [S, E], F32, isOutput=False)
    Vd = nc.declare_dram_parameter("V", [S, E], F32, isOutput=False)
    Wqd = nc.declare_dram_parameter("Wq", [E, H], F32, isOutput=False)
    Wkd = nc.declare_dram_parameter("Wk", [E, H], F32, isOutput=False)
    Wvd = nc.declare_dram_parameter("Wv", [E, H], F32, isOutput=False)
    bqd = nc.declare_dram_parameter("bq", [H], F32, isOutput=False)
    bkd = nc.declare_dram_parameter("bk", [H], F32, isOutput=False)
    bvd = nc.declare_dram_parameter("bv", [H], F32, isOutput=False)
    outd = nc.declare_dram_parameter("out", [S, H], F32, isOutput=True)

    with tile.TileContext(nc) as tc, ExitStack() as ctx:
        persist = ctx.enter_context(tc.tile_pool(name="persist", bufs=1))
        xnat_p = ctx.enter_context(tc.tile_pool(name="xnat", bufs=6))
        xt_p = ctx.enter_context(tc.tile_pool(name="xt", bufs=12))
        ew_p = ctx.enter_context(tc.tile_pool(name="ew", bufs=4))
        small_p = ctx.enter_context(tc.tile_pool(name="small", bufs=4))

        ps_tp = ctx.enter_context(tc.tile_pool(name="ps_tp", bufs=2, space="PSUM"))
        ps_mm = ctx.enter_context(tc.tile_pool(name="ps_mm", bufs=2, space="PSUM"))
        ps_outT = ctx.enter_context(tc.tile_pool(name="ps_outT", bufs=2, space="PSUM"))
        ps_sums = ctx.enter_context(tc.tile_pool(name="ps_sums", bufs=1, space="PSUM"))

        # ---- constants ----
        ident = persist.tile([PT, PT], F32, tag="ident")
        make_identity(nc, ident)
        ident_r = persist.tile([PT, PT], MMD, tag="ident_r")
        nc.vector.tensor_copy(ident_r, ident)
        ones_f = persist.tile([PT, 1], F32, tag="ones_f")
        nc.gpsimd.memset(ones_f, 1.0)
        ones_col = persist.tile([PT, 1], MMD, tag="ones_col")
        nc.vector.tensor_copy(ones_col, ones_f)
        one_1 = persist.tile([1, 1], F32, tag="one_1")
        nc.gpsimd.memset(one_1, 1.0)

        masks = []
        for m in range(TPC):
            mk = persist.tile([PT, CH], F32, tag=f"mask{m}")
            nc.gpsimd.memset(mk, 1.0)
            # keep (=1.0) where f - p - 128*m >= 0 else fill 0.0
            nc.gpsimd.affine_select(
                out=mk, in_=mk, compare_op=mybir.AluOpType.is_ge,
                fill=0.0, base=-PT * m, pattern=[[1, CH]], channel_multiplier=-1,
            )
            masks.append(mk)

        wts = {}
        for nm, d in (("wq", Wqd), ("wk", Wkd), ("wv", Wvd)):
            wf = persist.tile([PT, EC, H], F32, tag=nm + "f")
            nc.gpsimd.dma_start(out=wf, in_=d[:].rearrange("(c p) h -> p c h", p=PT))
            w = persist.tile([PT, EC, H], MMD, tag=nm)
            nc.vector.tensor_copy(w, wf)
            wts[nm] = w
        bias = {}
        for nm, d in (("bq", bqd), ("bk", bkd), ("bv", bvd)):
            b = persist.tile([H, 1], F32, tag=nm)
            nc.gpsimd.dma_start(out=b, in_=d[:].unsqueeze(1))
            bias[nm] = b

        # persistent projected tensors (fp32r, written by rounding-capable ops)
        qT = [persist.tile([H, CH], MMD, tag=f"qT{c}", name=f"qT{c}")
              for c in range(NCHUNK)]
        kT = [persist.tile([H, CH], MMD, tag=f"kT{c}", name=f"kT{c}")
              for c in range(NCHUNK)]
        vnat = [persist.tile([PT, H], MMD, tag=f"v{j}", name=f"v{j}")
                for j in range(S // PT)]

        # ---- phase 1: load, transpose (fp32 exact), project (fp32r) ----
        def do_input(Xd, wname, bname, out_tiles, is_v):
            w = wts[wname]
            b = bias[bname]
            for c in range(NCHUNK):
                xts = []
                nats = []
                for t in range(TPC):
                    xn = xnat_p.tile([PT, E], F32, tag="xn")
                    nc.sync.dma_start(
                        out=xn, in_=Xd[c * CH + t * PT: c * CH + (t + 1) * PT, :])
                    nats.append(xn)
                for e in range(EC):
                    tp = ps_tp.tile([PT, CH], F32, tag="tp")
                    for t in range(TPC):
                        nc.tensor.transpose(
                            out=tp[:, t * PT:(t + 1) * PT],
                            in_=nats[t][:, e * PT:(e + 1) * PT],
                            identity=ident[:],
                        )
                    xt = xt_p.tile([PT, CH], MMD, tag="xt")
                    if e % 2 == 0:
                        nc.scalar.copy(out=xt, in_=tp)
                    else:
                        nc.vector.tensor_copy(xt, tp)
                    xts.append(xt)
                # projection: pT_chunk [H, CH] += w_e^T.T @ xt_e
                pj = ps_mm.tile([H, CH], F32, tag="mm")
                for e in range(EC):
                    nc.tensor.matmul(pj, w[:, e, :], xts[e],
                                     start=(e == 0), stop=(e == EC - 1))
                if not is_v:
                    nc.scalar.activation(out=out_tiles[c], in_=pj,
                                         func=mybir.ActivationFunctionType.Identity,
                                         bias=b[:], scale=1.0)
                else:
                    vT = small_p.tile([H, CH], MMD, tag="vT")
                    nc.scalar.activation(out=vT, in_=pj,
                                         func=mybir.ActivationFunctionType.Identity,
                                         bias=b[:], scale=1.0)
                    vtp = ps_mm.tile([PT, CH], MMD, tag="mm")
                    for t in range(TPC):
                        nc.tensor.transpose(
                            out=vtp[:, t * PT:(t + 1) * PT],
                            in_=vT[:, t * PT:(t + 1) * PT],
                            identity=ident_r[:],
                        )
                    for t in range(TPC):
                        nc.vector.tensor_copy(out_tiles[c * TPC + t],
                                              vtp[:, t * PT:(t + 1) * PT])

        do_input(Kd, "wk", "bk", kT, False)
        do_input(Qd, "wq", "bq", qT, False)
        do_input(Vd, "wv", "bv", vnat, True)

        # ---- phase 2: attention per Sq chunk ----
        for c in range(NCHUNK):
            nk = (c + 1) * TPC   # causal: Sk tiles 0..nk-1
            oT = ps_outT.tile([H, CH], F32, tag="outT")
            sums = ps_sums.tile([1, CH], F32, tag="sums")
            for j in range(nk):
                wp = ps_mm.tile([PT, CH], F32, tag="mm")
                kc, kt = divmod(j, TPC)
                nc.tensor.matmul(wp, kT[kc][:, kt * PT:(kt + 1) * PT],
                                 qT[c], start=True, stop=True)
                ew = ew_p.tile([PT, CH], MMD, tag="ew")
                nc.scalar.activation(out=ew, in_=wp,
                                     func=mybir.ActivationFunctionType.Exp,
                                     scale=scale)
                m = j - c * TPC
                if m >= 0:
                    nc.vector.tensor_mul(ew, ew, masks[m])
                nc.tensor.matmul(sums, ones_col[:], ew,
                                 start=(j == 0), stop=(j == nk - 1))
                nc.tensor.matmul(oT, vnat[j][:], ew,
                                 start=(j == 0), stop=(j == nk - 1))

            # denominators: sums [1, CH] -> sumsT [128, TPC] -> recip
            sums_sb = small_p.tile([1, CH], F32, tag="sums_sb")
            nc.scalar.copy(out=sums_sb, in_=sums)
            sumsT = ps_sums.tile([PT, TPC], F32, tag="sumsT")
            for t in range(TPC):
                nc.tensor.matmul(sumsT[:, t:t + 1],
                                 sums_sb[0:1, t * PT:(t + 1) * PT],
                                 one_1[:], start=True, stop=True)
            recip = small_p.tile([PT, TPC], F32, tag="recip")
            nc.vector.reciprocal(recip, sumsT[:, 0:TPC])

            # transpose outT back to [Sq, H], scale, store
            oT_sb = small_p.tile([H, CH], F32, tag="oT_sb")
            nc.scalar.copy(out=oT_sb, in_=oT)
            otp = ps_tp.tile([PT, CH], F32, tag="tp")
            for t in range(TPC):
                nc.tensor.transpose(out=otp[:, t * PT:(t + 1) * PT],
                                    in_=oT_sb[:, t * PT:(t + 1) * PT],
                                    identity=ident[:])
            for t in range(TPC):
                ob = small_p.tile([PT, H], F32, tag="ob")
                nc.scalar.activation(out=ob, in_=otp[:, t * PT:(t + 1) * PT],
                                     func=mybir.ActivationFunctionType.Identity,
                                     scale=recip[:, t:t + 1], bias=0.0)
                nc.sync.dma_start(
                    out=outd[c * CH + t * PT: c * CH + (t + 1) * PT, :], in_=ob)

    nc.compile()
    return nc




_NC_CACHE = {}


def _get_nc():
    if "nc" not in _NC_CACHE:
        _NC_CACHE["nc"] = build(S=2048, E=1024, H=128, n_cores=8, mm_r=True)
    return _NC_CACHE["nc"]


def kernel(Q, K, V, mask=None, Wq=None, bq=None, Wk=None, bk=None,
           Wv=None, bv=None, **_):
    """Full-input entry point: Q/K/V [8, 2048, 1024] fp32 -> out [8, 2048, 128].

    Data-parallel over batch: core i computes batch element i. The causal
    mask input is ignored (causality is hardcoded in the kernel structure).
    """
    from concourse.bass_utils import run_bass_kernel_spmd

    B = Q.shape[0]
    nc = _get_nc()
    f32 = np.float32
    in_maps = []
    for i in range(B):
        in_maps.append({
            "Q": np.ascontiguousarray(Q[i], dtype=f32),
            "K": np.ascontiguousarray(K[i], dtype=f32),
            "V": np.ascontiguousarray(V[i], dtype=f32),
            "Wq": np.ascontiguousarray(Wq, dtype=f32),
            "Wk": np.ascontiguousarray(Wk, dtype=f32),
            "Wv": np.ascontiguousarray(Wv, dtype=f32),
            "bq": np.ascontiguousarray(bq, dtype=f32),
            "bk": np.ascontiguousarray(bk, dtype=f32),
            "bv": np.ascontiguousarray(bv, dtype=f32),
        })
    r = run_bass_kernel_spmd(nc, in_maps, list(range(B)))
    return np.stack([r.results[i]["out"] for i in range(B)]).astype(np.float32)


# revision 3
# speedup vs baseline: 1.5615x; 1.5615x over previous
"""Causal single-head attention kernel for TRN2 (one batch element per core).

Computes: out = softmax(causal((X_q Wq + bq)(X_k Wk + bk)^T / sqrt(H))) (X_v Wv + bv)
Shapes per core: Q,K,V [S, E]; Wq/Wk/Wv [E, H]; bq/bk/bv [H]; out [S, H].

Layout strategy (per core):
- Inputs transposed on PE (fp32, exact) so E lands on partitions.
- Projections produce qT/kT/vT [H, S]; fp32r matmuls (1 cyc/row).
- Scores computed transposed: weiT [Sk, Sq-chunk]; exp+scale fused on ACT;
  causal handled by skipping above-diagonal tiles + 0/1 masks on diagonal.
- PV accumulates outT [H, Sq-chunk] in PSUM over Sk tiles; softmax
  denominators via ones-vector matmuls; normalization deferred to the end.
"""

from contextlib import ExitStack

import numpy as np

import concourse.bacc as bacc
import concourse.bass as bass
import concourse.mybir as mybir
import concourse.tile as tile
from concourse.masks import make_identity

F32 = mybir.dt.float32
F32R = mybir.dt.float32r
BF16 = mybir.dt.bfloat16

CH = 512          # Sq chunk width (psum bank)
PT = 128          # partition tile


def build(S=2048, E=1024, H=128, n_cores=8, mm_r=True):
    """Build + compile the Bacc kernel. Returns nc."""
    EC = E // PT              # E chunks (8)
    NCHUNK = S // CH          # Sq chunks (4)
    TPC = CH // PT            # S-tiles per chunk (4)
    scale = float(H) ** -0.5
    MMD = BF16                    # dtype for matmul operands

    nc = bacc.Bacc("TRN2", target_bir_lowering=False, debug=False,
                   num_devices=n_cores)

    Qd = nc.declare_dram_parameter("Q", [S, E], F32, isOutput=False)
    Kd = nc.declare_dram_parameter("K", [S, E], F32, isOutput=False)
    Vd = nc.declare_dram_parameter("V", [S, E], F32, isOutput=False)
    Wqd = nc.declare_dram_parameter("Wq", [E, H], F32, isOutput=False)
    Wkd = nc.declare_dram_parameter("Wk", [E, H], F32, isOutput=False)
    Wvd = nc.declare_dram_parameter("Wv", [E, H], F32, isOutput=False)
    bqd = nc.declare_dram_parameter("bq", [H], F32, isOutput=False)
    bkd = nc.declare_dram_parameter("bk", [H], F32, isOutput=False)
    bvd = nc.declare_dram_parameter("bv", [H], F32, isOutput=False)
    outd = nc.declare_dram_parameter("out", [S, H], F32, isOutput=True)

    with tile.TileContext(nc) as tc, ExitStack() as ctx:
        persist = ctx.enter_context(tc.tile_pool(name="persist", bufs=1))
        xnat_p = ctx.enter_context(tc.tile_pool(name="xnat", bufs=3))
        xt_p = ctx.enter_context(tc.tile_pool(name="xt", bufs=12))
        ew_p = ctx.enter_context(tc.tile_pool(name="ew", bufs=4))
        small_p = ctx.enter_context(tc.tile_pool(name="small", bufs=4))

        ps_tp = ctx.enter_context(tc.tile_pool(name="ps_tp", bufs=2, space="PSUM"))
        ps_mm = ctx.enter_context(tc.tile_pool(name="ps_mm", bufs=2, space="PSUM"))
        ps_outT = ctx.enter_context(tc.tile_pool(name="ps_outT", bufs=2, space="PSUM"))
        ps_sums = ctx.enter_context(tc.tile_pool(name="ps_sums", bufs=1, space="PSUM"))

        # ---- constants ----
        ident = persist.tile([PT, PT], F32, tag="ident")
        make_identity(nc, ident)
        ident_b = persist.tile([PT, PT], MMD, tag="ident_b")
        make_identity(nc, ident_b)
        ones_col = persist.tile([PT, 1], MMD, tag="ones_col")
        nc.gpsimd.memset(ones_col, 1.0)
        one_1 = persist.tile([1, 1], F32, tag="one_1")
        nc.gpsimd.memset(one_1, 1.0)

        masks = []
        for m in range(TPC):
            mk = persist.tile([PT, CH], MMD, tag=f"mask{m}")
            nc.gpsimd.memset(mk, 1.0)
            # keep (=1.0) where f - p - 128*m >= 0 else fill 0.0
            nc.gpsimd.affine_select(
                out=mk, in_=mk, compare_op=mybir.AluOpType.is_ge,
                fill=0.0, base=-PT * m, pattern=[[1, CH]], channel_multiplier=-1,
            )
            masks.append(mk)

        wts = {}
        for nm, d in (("wq", Wqd), ("wk", Wkd), ("wv", Wvd)):
            w = persist.tile([PT, EC, H], MMD, tag=nm)
            nc.gpsimd.dma_start(out=w, in_=d[:].rearrange("(c p) h -> p c h", p=PT))
            wts[nm] = w
        bias = {}
        for nm, d in (("bq", bqd), ("bk", bkd), ("bv", bvd)):
            b = persist.tile([H, 1], F32, tag=nm)
            nc.gpsimd.dma_start(out=b, in_=d[:].unsqueeze(1))
            bias[nm] = b

        # persistent projected tensors (fp32r, written by rounding-capable ops)
        qT = [persist.tile([H, CH], MMD, tag=f"qT{c}", name=f"qT{c}")
              for c in range(NCHUNK)]
        kT = [persist.tile([H, CH], MMD, tag=f"kT{c}", name=f"kT{c}")
              for c in range(NCHUNK)]
        vnat = [persist.tile([PT, H], MMD, tag=f"v{j}", name=f"v{j}")
                for j in range(S // PT)]

        # ---- phase 1: load, transpose (fp32 exact), project (fp32r) ----
        def do_input(Xd, wname, bname, out_tiles, is_v):
            w = wts[wname]
            b = bias[bname]
            for c in range(NCHUNK):
                xts = []
                xn = xnat_p.tile([PT, TPC, E], MMD, tag="xn")
                nc.gpsimd.dma_start(
                    out=xn, in_=Xd[c * CH:(c + 1) * CH, :].rearrange(
                        "(t p) e -> p t e", p=PT))
                for e in range(EC):
                    tp = ps_tp.tile([PT, CH], MMD, tag="tp")
                    for t in range(TPC):
                        nc.tensor.transpose(
                            out=tp[:, t * PT:(t + 1) * PT],
                            in_=xn[:, t, e * PT:(e + 1) * PT],
                            identity=ident_b[:],
                        )
                    xt = xt_p.tile([PT, CH], MMD, tag="xt")
                    if e % 2 == 0:
                        nc.scalar.copy(out=xt, in_=tp)
                    else:
                        nc.vector.tensor_copy(xt, tp)
                    xts.append(xt)
                # projection: pT_chunk [H, CH] += w_e^T.T @ xt_e
                pj = ps_mm.tile([H, CH], F32, tag="mm")
                for e in range(EC):
                    nc.tensor.matmul(pj, w[:, e, :], xts[e],
                                     start=(e == 0), stop=(e == EC - 1))
                if not is_v:
                    nc.vector.tensor_scalar_add(out_tiles[c], pj, b[:])
                else:
                    vT = small_p.tile([H, CH], MMD, tag="vT")
                    nc.vector.tensor_scalar_add(vT, pj, b[:])
                    vtp = ps_mm.tile([PT, CH], MMD, tag="mm")
                    for t in range(TPC):
                        nc.tensor.transpose(
                            out=vtp[:, t * PT:(t + 1) * PT],
                            in_=vT[:, t * PT:(t + 1) * PT],
                            identity=ident_b[:],
                        )
                    for t in range(TPC):
                        nc.vector.tensor_copy(out_tiles[c * TPC + t],
                                              vtp[:, t * PT:(t + 1) * PT])

        do_input(Kd, "wk", "bk", kT, False)
        do_input(Qd, "wq", "bq", qT, False)
        do_input(Vd, "wv", "bv", vnat, True)

        # ---- phase 2: attention per Sq chunk ----
        for c in range(NCHUNK):
            nk = (c + 1) * TPC   # causal: Sk tiles 0..nk-1
            oT = ps_outT.tile([H, CH], F32, tag="outT")
            sums = ps_sums.tile([1, CH], F32, tag="sums")
            for j in range(nk):
                wp = ps_mm.tile([PT, CH], F32, tag="mm")
                kc, kt = divmod(j, TPC)
                nc.tensor.matmul(wp, kT[kc][:, kt * PT:(kt + 1) * PT],
                                 qT[c], start=True, stop=True)
                ew = ew_p.tile([PT, CH], MMD, tag="ew")
                nc.scalar.activation(out=ew, in_=wp,
                                     func=mybir.ActivationFunctionType.Exp,
                                     scale=scale)
                m = j - c * TPC
                if m >= 0:
                    nc.vector.tensor_mul(ew, ew, masks[m])
                nc.tensor.matmul(sums, ones_col[:], ew,
                                 start=(j == 0), stop=(j == nk - 1))
                nc.tensor.matmul(oT, vnat[j][:], ew,
                                 start=(j == 0), stop=(j == nk - 1))

            # denominators: sums [1, CH] -> sumsT [128, TPC] -> recip
            sums_sb = small_p.tile([1, CH], F32, tag="sums_sb")
            nc.vector.tensor_copy(sums_sb, sums)
            sumsT = ps_sums.tile([PT, TPC], F32, tag="sumsT")
            for t in range(TPC):
                nc.tensor.matmul(sumsT[:, t:t + 1],
                                 sums_sb[0:1, t * PT:(t + 1) * PT],
                                 one_1[:], start=True, stop=True)
            recip = small_p.tile([PT, TPC], F32, tag="recip")
            nc.vector.reciprocal(recip, sumsT[:, 0:TPC])

            # transpose outT back to [Sq, H], scale, store
            oT_sb = small_p.tile([H, CH], F32, tag="oT_sb")
            nc.vector.tensor_copy(oT_sb, oT)
            otp = ps_tp.tile([PT, CH], F32, tag="tp")
            for t in range(TPC):
                nc.tensor.transpose(out=otp[:, t * PT:(t + 1) * PT],
                                    in_=oT_sb[:, t * PT:(t + 1) * PT],
                                    identity=ident[:])
            for t in range(TPC):
                ob = small_p.tile([PT, H], F32, tag="ob")
                nc.vector.tensor_scalar_mul(ob, otp[:, t * PT:(t + 1) * PT],
                                            recip[:, t:t + 1])
                nc.sync.dma_start(
                    out=outd[c * CH + t * PT: c * CH + (t + 1) * PT, :], in_=ob)

    nc.compile()
    return nc




_NC_CACHE = {}


def _get_nc():
    if "nc" not in _NC_CACHE:
        _NC_CACHE["nc"] = build(S=2048, E=1024, H=128, n_cores=8, mm_r=True)
    return _NC_CACHE["nc"]


def kernel(Q, K, V, mask=None, Wq=None, bq=None, Wk=None, bk=None,
           Wv=None, bv=None, **_):
    """Full-input entry point: Q/K/V [8, 2048, 1024] fp32 -> out [8, 2048, 128].

    Data-parallel over batch: core i computes batch element i. The causal
    mask input is ignored (causality is hardcoded in the kernel structure).
    """
    from concourse.bass_utils import run_bass_kernel_spmd

    B = Q.shape[0]
    nc = _get_nc()
    f32 = np.float32
    in_maps = []
    for i in range(B):
        in_maps.append({
            "Q": np.ascontiguousarray(Q[i], dtype=f32),
            "K": np.ascontiguousarray(K[i], dtype=f32),
            "V": np.ascontiguousarray(V[i], dtype=f32),
            "Wq": np.ascontiguousarray(Wq, dtype=f32),
            "Wk": np.ascontiguousarray(Wk, dtype=f32),
            "Wv": np.ascontiguousarray(Wv, dtype=f32),
            "bq": np.ascontiguousarray(bq, dtype=f32),
            "bk": np.ascontiguousarray(bk, dtype=f32),
            "bv": np.ascontiguousarray(bv, dtype=f32),
        })
    r = run_bass_kernel_spmd(nc, in_maps, list(range(B)))
    return np.stack([r.results[i]["out"] for i in range(B)]).astype(np.float32)


# revision 4
# speedup vs baseline: 1.6281x; 1.0427x over previous
"""Causal single-head attention kernel for TRN2 (one batch element per core).

Computes: out = softmax(causal((X_q Wq + bq)(X_k Wk + bk)^T / sqrt(H))) (X_v Wv + bv)
Shapes per core: Q,K,V [S, E]; Wq/Wk/Wv [E, H]; bq/bk/bv [H]; out [S, H].

Layout strategy (per core):
- Inputs transposed on PE (fp32, exact) so E lands on partitions.
- Projections produce qT/kT/vT [H, S]; fp32r matmuls (1 cyc/row).
- Scores computed transposed: weiT [Sk, Sq-chunk]; exp+scale fused on ACT;
  causal handled by skipping above-diagonal tiles + 0/1 masks on diagonal.
- PV accumulates outT [H, Sq-chunk] in PSUM over Sk tiles; softmax
  denominators via ones-vector matmuls; normalization deferred to the end.
"""

from contextlib import ExitStack

import numpy as np

import concourse.bacc as bacc
import concourse.bass as bass
import concourse.mybir as mybir
import concourse.tile as tile
from concourse.masks import make_identity

F32 = mybir.dt.float32
F32R = mybir.dt.float32r
BF16 = mybir.dt.bfloat16

CH = 512          # Sq chunk width (psum bank)
PT = 128          # partition tile


def build(S=2048, E=1024, H=128, n_cores=8, mm_r=True):
    """Build + compile the Bacc kernel. Returns nc."""
    EC = E // PT              # E chunks (8)
    NCHUNK = S // CH          # Sq chunks (4)
    TPC = CH // PT            # S-tiles per chunk (4)
    scale = float(H) ** -0.5
    MMD = BF16                    # dtype for matmul operands

    nc = bacc.Bacc("TRN2", target_bir_lowering=False, debug=False,
                   num_devices=n_cores)

    Qd = nc.declare_dram_parameter("Q", [S, E], F32, isOutput=False)
    Kd = nc.declare_dram_parameter("K", [S, E], F32, isOutput=False)
    Vd = nc.declare_dram_parameter("V", [S, E], F32, isOutput=False)
    Wqd = nc.declare_dram_parameter("Wq", [E, H], F32, isOutput=False)
    Wkd = nc.declare_dram_parameter("Wk", [E, H], F32, isOutput=False)
    Wvd = nc.declare_dram_parameter("Wv", [E, H], F32, isOutput=False)
    bqd = nc.declare_dram_parameter("bq", [H], F32, isOutput=False)
    bkd = nc.declare_dram_parameter("bk", [H], F32, isOutput=False)
    bvd = nc.declare_dram_parameter("bv", [H], F32, isOutput=False)
    outd = nc.declare_dram_parameter("out", [S, H], F32, isOutput=True)

    with tile.TileContext(nc) as tc, ExitStack() as ctx:
        persist = ctx.enter_context(tc.tile_pool(name="persist", bufs=1))
        xnat_p = ctx.enter_context(tc.tile_pool(name="xnat", bufs=3))
        xt_p = ctx.enter_context(tc.tile_pool(name="xt", bufs=12))
        ew_p = ctx.enter_context(tc.tile_pool(name="ew", bufs=6))
        small_p = ctx.enter_context(tc.tile_pool(name="small", bufs=4))

        ps_tp = ctx.enter_context(tc.tile_pool(name="ps_tp", bufs=2, space="PSUM"))
        ps_mm = ctx.enter_context(tc.tile_pool(name="ps_mm", bufs=3, space="PSUM"))
        ps_outT = ctx.enter_context(tc.tile_pool(name="ps_outT", bufs=1, space="PSUM"))
        ps_sums = ctx.enter_context(tc.tile_pool(name="ps_sums", bufs=1, space="PSUM"))

        # ---- constants ----
        ident = persist.tile([PT, PT], F32, tag="ident")
        make_identity(nc, ident)
        ident_b = persist.tile([PT, PT], MMD, tag="ident_b")
        make_identity(nc, ident_b)
        ones_col = persist.tile([PT, 1], MMD, tag="ones_col")
        nc.gpsimd.memset(ones_col, 1.0)
        one_1 = persist.tile([1, 1], F32, tag="one_1")
        nc.gpsimd.memset(one_1, 1.0)

        masks = []
        for m in range(TPC):
            mk = persist.tile([PT, CH], MMD, tag=f"mask{m}")
            nc.gpsimd.memset(mk, 1.0)
            # keep (=1.0) where f - p - 128*m >= 0 else fill 0.0
            nc.gpsimd.affine_select(
                out=mk, in_=mk, compare_op=mybir.AluOpType.is_ge,
                fill=0.0, base=-PT * m, pattern=[[1, CH]], channel_multiplier=-1,
            )
            masks.append(mk)

        wts = {}
        for nm, d in (("wq", Wqd), ("wk", Wkd), ("wv", Wvd)):
            w = persist.tile([PT, EC, H], MMD, tag=nm)
            nc.gpsimd.dma_start(out=w, in_=d[:].rearrange("(c p) h -> p c h", p=PT))
            wts[nm] = w
        bias = {}
        for nm, d in (("bq", bqd), ("bk", bkd), ("bv", bvd)):
            b = persist.tile([H, 1], F32, tag=nm)
            nc.gpsimd.dma_start(out=b, in_=d[:].unsqueeze(1))
            bias[nm] = b

        # persistent projected tensors (fp32r, written by rounding-capable ops)
        qT = [persist.tile([H, CH], MMD, tag=f"qT{c}", name=f"qT{c}")
              for c in range(NCHUNK)]
        kT = [persist.tile([H, CH], MMD, tag=f"kT{c}", name=f"kT{c}")
              for c in range(NCHUNK)]
        vnat = [persist.tile([PT, H], MMD, tag=f"v{j}", name=f"v{j}")
                for j in range(S // PT)]

        # ---- phase 1: load, transpose (fp32 exact), project (fp32r) ----
        def do_input(Xd, wname, bname, out_tiles, is_v):
            w = wts[wname]
            b = bias[bname]
            for c in range(NCHUNK):
                xts = []
                xn = xnat_p.tile([PT, TPC, E], MMD, tag="xn")
                nc.gpsimd.dma_start(
                    out=xn, in_=Xd[c * CH:(c + 1) * CH, :].rearrange(
                        "(t p) e -> p t e", p=PT))
                for e in range(EC):
                    tp = ps_tp.tile([PT, CH], MMD, tag="tp")
                    for t in range(TPC):
                        nc.tensor.transpose(
                            out=tp[:, t * PT:(t + 1) * PT],
                            in_=xn[:, t, e * PT:(e + 1) * PT],
                            identity=ident_b[:],
                        )
                    xt = xt_p.tile([PT, CH], MMD, tag="xt")
                    if e % 2 == 0:
                        nc.scalar.copy(out=xt, in_=tp)
                    else:
                        nc.vector.tensor_copy(xt, tp)
                    xts.append(xt)
                # projection: pT_chunk [H, CH] += w_e^T.T @ xt_e
                pj = ps_mm.tile([H, CH], F32, tag="mm")
                for e in range(EC):
                    nc.tensor.matmul(pj, w[:, e, :], xts[e],
                                     start=(e == 0), stop=(e == EC - 1))
                if not is_v:
                    nc.vector.tensor_scalar_add(out_tiles[c], pj, b[:])
                else:
                    vT = small_p.tile([H, CH], MMD, tag="vT")
                    nc.vector.tensor_scalar_add(vT, pj, b[:])
                    vtp = ps_mm.tile([PT, CH], MMD, tag="mm")
                    for t in range(TPC):
                        nc.tensor.transpose(
                            out=vtp[:, t * PT:(t + 1) * PT],
                            in_=vT[:, t * PT:(t + 1) * PT],
                            identity=ident_b[:],
                        )
                    for t in range(TPC):
                        nc.vector.tensor_copy(out_tiles[c * TPC + t],
                                              vtp[:, t * PT:(t + 1) * PT])

        do_input(Kd, "wk", "bk", kT, False)
        do_input(Qd, "wq", "bq", qT, False)
        do_input(Vd, "wv", "bv", vnat, True)

        # ---- phase 2: attention per Sq chunk ----
        for c in range(NCHUNK):
            nk = (c + 1) * TPC   # causal: Sk tiles 0..nk-1
            oT = ps_outT.tile([H, CH], F32, tag="outT")
            sums = ps_sums.tile([1, CH], F32, tag="sums")
            for j in range(nk):
                wp = ps_mm.tile([PT, CH], F32, tag="mm")
                kc, kt = divmod(j, TPC)
                nc.tensor.matmul(wp, kT[kc][:, kt * PT:(kt + 1) * PT],
                                 qT[c], start=True, stop=True)
                ew = ew_p.tile([PT, CH], MMD, tag="ew")
                nc.scalar.activation(out=ew, in_=wp,
                                     func=mybir.ActivationFunctionType.Exp,
                                     scale=scale)
                m = j - c * TPC
                if m >= 0:
                    nc.vector.tensor_mul(ew, ew, masks[m])
                nc.tensor.matmul(sums, ones_col[:], ew,
                                 start=(j == 0), stop=(j == nk - 1))
                nc.tensor.matmul(oT, vnat[j][:], ew,
                                 start=(j == 0), stop=(j == nk - 1))

            # denominators: sums [1, CH] -> sumsT [128, TPC] -> recip
            sums_sb = small_p.tile([1, CH], F32, tag="sums_sb")
            nc.vector.tensor_copy(sums_sb, sums)
            sumsT = ps_sums.tile([PT, TPC], F32, tag="sumsT")
            for t in range(TPC):
                nc.tensor.matmul(sumsT[:, t:t + 1],
                                 sums_sb[0:1, t * PT:(t + 1) * PT],
                                 one_1[:], start=True, stop=True)
            recip = small_p.tile([PT, TPC], F32, tag="recip")
            nc.vector.reciprocal(recip, sumsT[:, 0:TPC])

            # transpose outT back to [Sq, H], scale, store
            oT_sb = small_p.tile([H, CH], F32, tag="oT_sb")
            nc.vector.tensor_copy(oT_sb, oT)
            otp = ps_tp.tile([PT, CH], F32, tag="tp")
            for t in range(TPC):
                nc.tensor.transpose(out=otp[:, t * PT:(t + 1) * PT],
                                    in_=oT_sb[:, t * PT:(t + 1) * PT],
                                    identity=ident[:])
            for t in range(TPC):
                ob = small_p.tile([PT, H], F32, tag="ob")
                nc.vector.tensor_scalar_mul(ob, otp[:, t * PT:(t + 1) * PT],
                                            recip[:, t:t + 1])
                nc.sync.dma_start(
                    out=outd[c * CH + t * PT: c * CH + (t + 1) * PT, :], in_=ob)

    nc.compile()
    return nc




_NC_CACHE = {}


def _get_nc():
    if "nc" not in _NC_CACHE:
        _NC_CACHE["nc"] = build(S=2048, E=1024, H=128, n_cores=8, mm_r=True)
    return _NC_CACHE["nc"]


def kernel(Q, K, V, mask=None, Wq=None, bq=None, Wk=None, bk=None,
           Wv=None, bv=None, **_):
    """Full-input entry point: Q/K/V [8, 2048, 1024] fp32 -> out [8, 2048, 128].

    Data-parallel over batch: core i computes batch element i. The causal
    mask input is ignored (causality is hardcoded in the kernel structure).
    """
    from concourse.bass_utils import run_bass_kernel_spmd

    B = Q.shape[0]
    nc = _get_nc()
    f32 = np.float32
    in_maps = []
    for i in range(B):
        in_maps.append({
            "Q": np.ascontiguousarray(Q[i], dtype=f32),
            "K": np.ascontiguousarray(K[i], dtype=f32),
            "V": np.ascontiguousarray(V[i], dtype=f32),
            "Wq": np.ascontiguousarray(Wq, dtype=f32),
            "Wk": np.ascontiguousarray(Wk, dtype=f32),
            "Wv": np.ascontiguousarray(Wv, dtype=f32),
            "bq": np.ascontiguousarray(bq, dtype=f32),
            "bk": np.ascontiguousarray(bk, dtype=f32),
            "bv": np.ascontiguousarray(bv, dtype=f32),
        })
    r = run_bass_kernel_spmd(nc, in_maps, list(range(B)))
    return np.stack([r.results[i]["out"] for i in range(B)]).astype(np.float32)
